# revision 24
# baseline (speedup 1.0000x reference)
"""Trainium2 Bass kernel: 7x7 valid cross-correlation + bias on a 4096x4096 f32 image.

Formulation: banded matmul on the TensorEngine.
  out[r, c] = sum_{di,dj} w[di,dj] * x[r+di, c+dj]
For an output row-strip of M=122 rows starting at r0, using K=128 input rows:
  out[r0+m, c] = sum_k A_dj[k, m] * x[r0+k, c+dj]   summed over dj=0..6
where A_dj[k, m] = w[k-m, dj] for 0 <= k-m < 7 (a banded [128, 122] matrix,
precomputed on host from the 49 kernel weights). The 7 dj-terms accumulate
into one PSUM bank via shifted column slices of the same SBUF rhs tile.

Matmuls run in bf16 (1 col/cycle on the PE vs 4 for fp32; fp32 PSUM accum);
the output is written back as bf16 and upcast on the host. Measured rel-err
vs the fp32 reference is ~3e-3, inside the 2e-2 gate.

DMA structure (measured on this platform):
  - each dma_start's packets go to only 2 of the 16 SDMA engines (~26 GB/s
    each); HWDGE SBUF->HBM DMAs always land on the same pair, while
    successive SWDGE (gpsimd) DMAs rotate pairs round-robin. HBM->SBUF
    loads DO spread across all 16.
  - concurrent DMAs share the 16 engines at packet granularity, so eager
    prefetch steals bandwidth from the urgent first loads; all input loads
    go on ONE queue (Sync) as 6 large in-order chunk DMAs (strip-major
    host-packed layout, contiguous per partition), band/bias on Scalar.
    Output stores are SWDGE, each PSUM group split into 4 partition-range
    sub-DMAs so rotating engine pairs carry it.
  - bias-add drains PSUM via tensor_tensor (1-port DVE mode; tensor_scalar
    can enter the 2-port mode that starves SWDGE descriptor generation).

Schedule: PSUM groups of <=4 strips; dj is the outer loop within a group so
matmuls sharing a stationary band matrix run back-to-back; PSUM bank reuse
distance is always >= 1 full group so matmuls never wait on a recent DVE
drain. Leading/trailing groups are 2 strips to shorten the pipeline
head/tail, and a few dummy matmuls on a zeroed scratch tile warm the PE
clock (HAM) while the first input chunk is still in flight.

Sharding: output columns are split across the 8 cores (512 cols/core);
each core processes all 4090 output rows. Kernel + bias replicated.
"""

import numpy as np

H, W = 4096, 4096
KH, KW = 7, 7
OH, OW = H - KH + 1, W - KW + 1  # 4090, 4090
N_CORES = 8
CW = 512               # output columns per core
IW = CW + KW - 1       # input columns per core (518)
STRIP = 122            # output rows per strip (K = STRIP + KH - 1 = 128)
MB = 128               # stationary block columns (M padded 122 -> 128)
N_STRIPS = (OH + STRIP - 1) // STRIP  # 34 (last strip M=64, K=70)
CH = 4                 # strips per chunk / PSUM group

_cache = {}


def _chunks():
    # [2, 2, 2, 2, 4, 4, 4, 4, 4, 2, 2, 1, 1]
    sizes = [2, 2, 2, 2] + [CH] * ((N_STRIPS - 14) // CH) + [2, 2, 1, 1]
    rem = N_STRIPS - sum(sizes)
    assert rem == 0, sizes
    out, s = [], 0
    for n in sizes:
        out.append(list(range(s, s + n)))
        s += n
    return out


def _build_nc():
    import concourse.bacc as bacc
    import concourse.mybir as mybir
    from concourse.tile import TileContext

    f32 = mybir.dt.float32
    bf16 = mybir.dt.bfloat16

    nc = bacc.Bacc("TRN2", target_bir_lowering=False, debug=False)
    xs = nc.dram_tensor("xs", [128, N_STRIPS * IW], bf16, kind="ExternalInput")
    bands = nc.dram_tensor("bands", [128, KW * MB], bf16, kind="ExternalInput")
    biasv = nc.dram_tensor("biasv", [128, 1], f32, kind="ExternalInput")
    # Packed output: out[m, s*CW + c] = out_full[122*s + m, c]; host unpacks.
    out = nc.dram_tensor("out", [STRIP, N_STRIPS * CW], bf16, kind="ExternalOutput")

    chunks = _chunks()

    with TileContext(nc) as tc:
        with (
            tc.tile_pool(name="const", bufs=1) as cpool,
            tc.tile_pool(name="rhs", bufs=6) as rpool,
            tc.tile_pool(name="obuf", bufs=4) as opool,
            tc.tile_pool(name="psum", bufs=8, space="PSUM") as ppool,
        ):
            # First input chunk on the (otherwise idle) Sync queue in
            # parallel with the band matrix on Scalar, so the first matmul's
            # dependencies land as early as possible; the remaining chunks
            # all prefetch behind them on Scalar.
            band_t = cpool.tile([128, KW * MB], bf16)
            nc.scalar.dma_start(out=band_t[:, :], in_=bands[:, :])
            # Input DMAs are decoupled from PSUM grouping: 6 large loads keep
            # the DMA semaphore-lane count low (lanes recycle across ~9 sems;
            # more dma_starts means later ones wait on unrelated receipts).
            in_sizes = [3, 6, 7, 7, 7, 4]
            assert sum(in_sizes) == N_STRIPS
            strip_tile = {}
            s0 = 0
            for ii, n in enumerate(in_sizes):
                xt = rpool.tile([128, max(in_sizes) * IW], bf16, tag="rhs")
                eng = nc.sync  # one queue: in-order completion at full BW
                eng.dma_start(
                    out=xt[:, : n * IW], in_=xs[:, s0 * IW : (s0 + n) * IW]
                )
                for j in range(n):
                    strip_tile[s0 + j] = (xt, j * IW)
                if ii == 0:
                    bias1_t = cpool.tile([128, 1], f32)
                    nc.scalar.dma_start(out=bias1_t[:, :], in_=biasv[:, :])
                s0 += n

            # HAM warm-up: the PE would otherwise sit idle for ~3us while
            # the first input chunk lands, then pay the 1.2GHz cold-clock
            # penalty for its first ~3.4us of real matmuls. A burst of dummy
            # matmuls on a memset scratch tile (never read back) keeps the
            # activity monitor busy so real matmuls start at 2.4GHz.
            warm_t = cpool.tile([128, 640], bf16)
            nc.vector.memset(warm_t[:, :], 0.0)
            # broadcast the 512B bias input to [128, CW] on-chip (cheaper than
            # shipping a 256KB constant ahead of the input chunks)
            bias_t = cpool.tile([128, CW], f32)
            nc.vector.tensor_scalar_add(
                bias_t[:, :], warm_t[:, :CW], bias1_t[:, :1]
            )
            warm_ps = ppool.tile([128, CW], f32, name="ps", tag="ps")
            for _ in range(3):
                nc.tensor.matmul(
                    warm_ps[:, :],
                    warm_t[:, :128],
                    warm_t[:, 128:640],
                    start=True,
                    stop=True,
                )

            psplits4 = [(0, 32), (32, 64), (64, 96), (96, STRIP)]
            psplits2 = [(0, 64), (64, STRIP)]
            for ci, strips in enumerate(chunks):
                s0, n = strips[0], len(strips)
                dims = []
                for s in strips:
                    r0 = s * STRIP
                    dims.append((r0, min(STRIP, OH - r0), min(128, H - r0)))
                ps_ts = [
                    ppool.tile([128, CW], f32, name="ps", tag="ps") for _ in strips
                ]
                for dj in range(KW):
                    lhsT = band_t[:, dj * MB : dj * MB + MB]
                    for j, (r0, M, K) in enumerate(dims):
                        sxt, soff = strip_tile[strips[j]]
                        nc.tensor.matmul(
                            ps_ts[j][:, :],
                            lhsT[:K, :],
                            sxt[:K, soff + dj : soff + dj + CW],
                            start=(dj == 0),
                            stop=(dj == KW - 1),
                        )
                ot = opool.tile([128, CH * CW], bf16, tag="ot")
                for j, (r0, M, K) in enumerate(dims):
                    nc.vector.tensor_tensor(
                        ot[:M, j * CW : (j + 1) * CW],
                        ps_ts[j][:M, :],
                        bias_t[:M, :],
                        mybir.AluOpType.add,
                    )
                # SWDGE write-out, split by partition range: each dma_start
                # streams on 2 SDMA engines and successive ones rotate pairs,
                # so 4 sub-DMAs put ~8 engines on one chunk.
                if ci >= len(chunks) - 2:
                    # tail groups (1 strip each): HWDGE store on the by-now
                    # idle Sync/Scalar queues, so the GpSimd queue's final
                    # drain doesn't wait on these receipts
                    eng = nc.sync if ci == len(chunks) - 1 else nc.scalar
                    eng.dma_start(
                        out=out[:, s0 * CW : (s0 + n) * CW],
                        in_=ot[:STRIP, : n * CW],
                    )
                else:
                    for p0, p1 in (psplits4 if n > 1 else psplits2):
                        nc.gpsimd.dma_start(
                            out=out[p0:p1, s0 * CW : (s0 + n) * CW],
                            in_=ot[p0:p1, : n * CW],
                        )

    nc.finalize()
    return nc


def _get_nc():
    if "nc" not in _cache:
        _cache["nc"] = _build_nc()
    return _cache["nc"]


def _build_bands(weight: np.ndarray) -> np.ndarray:
    """bands[k, dj*MB + m] = weight[k - m, dj] for 0 <= k-m < KH, m < STRIP."""
    w = np.asarray(weight, np.float32)
    bands = np.zeros((128, KW * MB), np.float32)
    m = np.arange(STRIP)
    for dj in range(KW):
        for di in range(KH):
            bands[m + di, dj * MB + m] = w[di, dj]
    return bands


def _prepare_in_maps(x, weight, bias):
    import ml_dtypes

    bf16 = ml_dtypes.bfloat16
    xb = np.ascontiguousarray(x, np.float32).astype(bf16)
    bands = _build_bands(weight).astype(bf16)
    bias_tile = np.full((128, 1), np.float32(np.asarray(bias).reshape(-1)[0]))

    # xs_packed[k, s, c] = x[122*s + k, c0 + c], zero beyond image edges.
    k_idx = np.arange(128)[:, None]
    s_idx = np.arange(N_STRIPS)[None, :]
    rows = k_idx + STRIP * s_idx  # [128, N_STRIPS]
    row_ok = rows < H
    rows_c = np.minimum(rows, H - 1)

    in_maps = []
    for c in range(N_CORES):
        c0 = c * CW
        avail = min(IW, W - c0)
        xsl = np.zeros((H, IW), bf16)
        xsl[:, :avail] = xb[:, c0 : c0 + avail]
        xs = xsl[rows_c, :]  # [128, N_STRIPS, IW]
        xs[~row_ok] = 0
        xs = np.ascontiguousarray(xs.reshape(128, N_STRIPS * IW))
        in_maps.append({"xs": xs, "bands": bands, "biasv": bias_tile})
    return in_maps


def _gather_out(per_core_outs) -> np.ndarray:
    out = np.empty((OH, OW), np.float32)
    for c in range(N_CORES):
        c0 = c * CW
        take = min(CW, OW - c0)
        po = per_core_outs[c]["out"].astype(np.float32).reshape(STRIP, N_STRIPS, CW)
        full = po.transpose(1, 0, 2).reshape(N_STRIPS * STRIP, CW)
        out[:, c0 : c0 + take] = full[:OH, :take]
    return out


def kernel(x: np.ndarray, weight: np.ndarray, bias: np.ndarray) -> np.ndarray:
    from concourse import bass_utils

    nc = _get_nc()
    in_maps = _prepare_in_maps(x, weight, bias)
    res = bass_utils.run_bass_kernel_spmd(nc, in_maps, list(range(N_CORES)))
    _cache["last_results"] = res
    return _gather_out(res.results)


# revision 25
# speedup vs baseline: 1.0508x; 1.0508x over previous
"""Trainium2 Bass kernel: 7x7 valid cross-correlation + bias on a 4096x4096 f32 image.

Formulation: banded matmul on the TensorEngine.
  out[r, c] = sum_{di,dj} w[di,dj] * x[r+di, c+dj]
For an output row-strip of M=122 rows starting at r0, using K=128 input rows:
  out[r0+m, c] = sum_k A_dj[k, m] * x[r0+k, c+dj]   summed over dj=0..6
where A_dj[k, m] = w[k-m, dj] for 0 <= k-m < 7 (a banded [128, 122] matrix,
precomputed on host from the 49 kernel weights). The 7 dj-terms accumulate
into one PSUM bank via shifted column slices of the same SBUF rhs tile.

Matmuls run in bf16 (1 col/cycle on the PE vs 4 for fp32; fp32 PSUM accum);
the output is written back as bf16 and upcast on the host. Measured rel-err
vs the fp32 reference is ~3e-3, inside the 2e-2 gate.

DMA structure (measured on this platform):
  - each dma_start's packets go to only 2 of the 16 SDMA engines (~26 GB/s
    each); HWDGE SBUF->HBM DMAs always land on the same pair, while
    successive SWDGE (gpsimd) DMAs rotate pairs round-robin. HBM->SBUF
    loads DO spread across all 16.
  - concurrent DMAs share the 16 engines at packet granularity, so eager
    prefetch steals bandwidth from the urgent first loads; all input loads
    go on ONE queue (Sync) as 6 large in-order chunk DMAs (strip-major
    host-packed layout, contiguous per partition), band/bias on Scalar.
    Output stores are SWDGE, each PSUM group split into 4 partition-range
    sub-DMAs so rotating engine pairs carry it.
  - bias-add drains PSUM via tensor_tensor (1-port DVE mode; tensor_scalar
    can enter the 2-port mode that starves SWDGE descriptor generation).

Schedule: PSUM groups of <=4 strips; dj is the outer loop within a group so
matmuls sharing a stationary band matrix run back-to-back; PSUM bank reuse
distance is always >= 1 full group so matmuls never wait on a recent DVE
drain. Leading/trailing groups are 2 strips to shorten the pipeline
head/tail, and a few dummy matmuls on a zeroed scratch tile warm the PE
clock (HAM) while the first input chunk is still in flight.

Sharding: output columns are split across the 8 cores (512 cols/core);
each core processes all 4090 output rows. Kernel + bias replicated.
"""

import numpy as np

H, W = 4096, 4096
KH, KW = 7, 7
OH, OW = H - KH + 1, W - KW + 1  # 4090, 4090
N_CORES = 8
CW = 512               # output columns per core
IW = CW + KW - 1       # input columns per core (518)
STRIP = 122            # output rows per strip (K = STRIP + KH - 1 = 128)
MB = 128               # stationary block columns (M padded 122 -> 128)
N_STRIPS = (OH + STRIP - 1) // STRIP  # 34 (last strip M=64, K=70)
CH = 4                 # strips per chunk / PSUM group

_cache = {}


def _chunks():
    # [2, 2, 2, 2, 4, 4, 4, 4, 4, 2, 2, 1, 1]
    sizes = [2, 2, 2, 2] + [CH] * ((N_STRIPS - 14) // CH) + [2, 2, 1, 1]
    rem = N_STRIPS - sum(sizes)
    assert rem == 0, sizes
    out, s = [], 0
    for n in sizes:
        out.append(list(range(s, s + n)))
        s += n
    return out


def _build_nc():
    import concourse.bacc as bacc
    import concourse.mybir as mybir
    from concourse.tile import TileContext

    f32 = mybir.dt.float32
    bf16 = mybir.dt.bfloat16

    nc = bacc.Bacc("TRN2", target_bir_lowering=False, debug=False)
    xs = nc.dram_tensor("xs", [128, N_STRIPS * IW], bf16, kind="ExternalInput")
    bands = nc.dram_tensor("bands", [128, KW * MB], bf16, kind="ExternalInput")
    biasv = nc.dram_tensor("biasv", [128, 1], f32, kind="ExternalInput")
    # Packed output: out[m, s*CW + c] = out_full[122*s + m, c]; host unpacks.
    out = nc.dram_tensor("out", [STRIP, N_STRIPS * CW], bf16, kind="ExternalOutput")

    chunks = _chunks()

    with TileContext(nc) as tc:
        with (
            tc.tile_pool(name="const", bufs=1) as cpool,
            tc.tile_pool(name="rhs", bufs=6) as rpool,
            tc.tile_pool(name="obuf", bufs=4) as opool,
            tc.tile_pool(name="psum", bufs=8, space="PSUM") as ppool,
        ):
            # First input chunk on the (otherwise idle) Sync queue in
            # parallel with the band matrix on Scalar, so the first matmul's
            # dependencies land as early as possible; the remaining chunks
            # all prefetch behind them on Scalar.
            band_t = cpool.tile([128, KW * MB], bf16)
            nc.scalar.dma_start(out=band_t[:, :], in_=bands[:, :])
            # Input DMAs are decoupled from PSUM grouping: 6 large loads keep
            # the DMA semaphore-lane count low (lanes recycle across ~9 sems;
            # more dma_starts means later ones wait on unrelated receipts).
            in_sizes = [3, 2, 4, 5, 7, 7, 6]
            assert sum(in_sizes) == N_STRIPS
            strip_tile = {}
            s0 = 0
            for ii, n in enumerate(in_sizes):
                xt = rpool.tile([128, max(in_sizes) * IW], bf16, tag="rhs")
                eng = nc.sync  # one queue: in-order completion at full BW
                eng.dma_start(
                    out=xt[:, : n * IW], in_=xs[:, s0 * IW : (s0 + n) * IW]
                )
                for j in range(n):
                    strip_tile[s0 + j] = (xt, j * IW)
                if ii == 0:
                    bias1_t = cpool.tile([128, 1], f32)
                    nc.scalar.dma_start(out=bias1_t[:, :], in_=biasv[:, :])
                s0 += n

            # HAM warm-up: the PE would otherwise sit idle for ~3us while
            # the first input chunk lands, then pay the 1.2GHz cold-clock
            # penalty for its first ~3.4us of real matmuls. A burst of dummy
            # matmuls on a memset scratch tile (never read back) keeps the
            # activity monitor busy so real matmuls start at 2.4GHz.
            warm_t = cpool.tile([128, 640], bf16)
            nc.vector.memset(warm_t[:, :], 0.0)
            # broadcast the 512B bias input to [128, CW] on-chip (cheaper than
            # shipping a 256KB constant ahead of the input chunks)
            bias_t = cpool.tile([128, CW], f32)
            nc.vector.tensor_scalar_add(
                bias_t[:, :], warm_t[:, :CW], bias1_t[:, :1]
            )
            warm_ps = ppool.tile([128, CW], f32, name="ps", tag="ps")
            for _ in range(3):
                nc.tensor.matmul(
                    warm_ps[:, :],
                    warm_t[:, :128],
                    warm_t[:, 128:640],
                    start=True,
                    stop=True,
                )

            psplits4 = [(0, 32), (32, 64), (64, 96), (96, STRIP)]
            psplits2 = [(0, 64), (64, STRIP)]
            for ci, strips in enumerate(chunks):
                s0, n = strips[0], len(strips)
                dims = []
                for s in strips:
                    r0 = s * STRIP
                    dims.append((r0, min(STRIP, OH - r0), min(128, H - r0)))
                ps_ts = [
                    ppool.tile([128, CW], f32, name="ps", tag="ps") for _ in strips
                ]
                for dj in range(KW):
                    lhsT = band_t[:, dj * MB : dj * MB + MB]
                    for j, (r0, M, K) in enumerate(dims):
                        sxt, soff = strip_tile[strips[j]]
                        nc.tensor.matmul(
                            ps_ts[j][:, :],
                            lhsT[:K, :],
                            sxt[:K, soff + dj : soff + dj + CW],
                            start=(dj == 0),
                            stop=(dj == KW - 1),
                        )
                ot = opool.tile([128, CH * CW], bf16, tag="ot")
                for j, (r0, M, K) in enumerate(dims):
                    nc.vector.tensor_tensor(
                        ot[:M, j * CW : (j + 1) * CW],
                        ps_ts[j][:M, :],
                        bias_t[:M, :],
                        mybir.AluOpType.add,
                    )
                # SWDGE write-out, split by partition range: each dma_start
                # streams on 2 SDMA engines and successive ones rotate pairs,
                # so 4 sub-DMAs put ~8 engines on one chunk.
                if ci == len(chunks) - 1:
                    # tail chunk: column-split per strip so the final sub-DMA
                    # depends only on the final strip's DVE drain
                    for j in range(n):
                        nc.gpsimd.dma_start(
                            out=out[:, (s0 + j) * CW : (s0 + j + 1) * CW],
                            in_=ot[:STRIP, j * CW : (j + 1) * CW],
                        )
                else:
                    for p0, p1 in (psplits4 if n > 1 else psplits2):
                        nc.gpsimd.dma_start(
                            out=out[p0:p1, s0 * CW : (s0 + n) * CW],
                            in_=ot[p0:p1, : n * CW],
                        )

    nc.finalize()
    return nc


def _get_nc():
    if "nc" not in _cache:
        _cache["nc"] = _build_nc()
    return _cache["nc"]


def _build_bands(weight: np.ndarray) -> np.ndarray:
    """bands[k, dj*MB + m] = weight[k - m, dj] for 0 <= k-m < KH, m < STRIP."""
    w = np.asarray(weight, np.float32)
    bands = np.zeros((128, KW * MB), np.float32)
    m = np.arange(STRIP)
    for dj in range(KW):
        for di in range(KH):
            bands[m + di, dj * MB + m] = w[di, dj]
    return bands


def _prepare_in_maps(x, weight, bias):
    import ml_dtypes

    bf16 = ml_dtypes.bfloat16
    xb = np.ascontiguousarray(x, np.float32).astype(bf16)
    bands = _build_bands(weight).astype(bf16)
    bias_tile = np.full((128, 1), np.float32(np.asarray(bias).reshape(-1)[0]))

    # xs_packed[k, s, c] = x[122*s + k, c0 + c], zero beyond image edges.
    k_idx = np.arange(128)[:, None]
    s_idx = np.arange(N_STRIPS)[None, :]
    rows = k_idx + STRIP * s_idx  # [128, N_STRIPS]
    row_ok = rows < H
    rows_c = np.minimum(rows, H - 1)

    in_maps = []
    for c in range(N_CORES):
        c0 = c * CW
        avail = min(IW, W - c0)
        xsl = np.zeros((H, IW), bf16)
        xsl[:, :avail] = xb[:, c0 : c0 + avail]
        xs = xsl[rows_c, :]  # [128, N_STRIPS, IW]
        xs[~row_ok] = 0
        xs = np.ascontiguousarray(xs.reshape(128, N_STRIPS * IW))
        in_maps.append({"xs": xs, "bands": bands, "biasv": bias_tile})
    return in_maps


def _gather_out(per_core_outs) -> np.ndarray:
    out = np.empty((OH, OW), np.float32)
    for c in range(N_CORES):
        c0 = c * CW
        take = min(CW, OW - c0)
        po = per_core_outs[c]["out"].astype(np.float32).reshape(STRIP, N_STRIPS, CW)
        full = po.transpose(1, 0, 2).reshape(N_STRIPS * STRIP, CW)
        out[:, c0 : c0 + take] = full[:OH, :take]
    return out


def kernel(x: np.ndarray, weight: np.ndarray, bias: np.ndarray) -> np.ndarray:
    from concourse import bass_utils

    nc = _get_nc()
    in_maps = _prepare_in_maps(x, weight, bias)
    res = bass_utils.run_bass_kernel_spmd(nc, in_maps, list(range(N_CORES)))
    _cache["last_results"] = res
    return _gather_out(res.results)
